# revision 9
# baseline (speedup 1.0000x reference)
"""Bass/TRN2 kernel for nn_EnvCollLoss (oriented-footprint raster collision loss).

Strategy: agents sharded by map index across 8 cores (2 cores per map); each
core keeps its map as a Y8-bitpacked fp16 ap_gather table in SBUF.

Per 128-state tile (all on-chip, no DMA in the loop):
  DVE computes footprint-point pixel indices; gpsimd ap_gather fetches the
  16-row word-pair per point on all 16 column-phase partitions of each group,
  plus a combined (column-phase one-hot x pair-half select) mask from a tiny
  table; gpsimd multiplies them; a block-diagonal [128,128] matmul reduces
  each 16-partition group so every partition holds its group's selected word;
  DVE pair-adds the PSUM halves; the scalar engine materializes a width-
  doubled copy; one DVE 32x32 stream-transpose + predicated merge lands the
  words state-major. dist = sqrt(min masked d2), so no collision-point
  reconstruction is needed. Heading normalization (pre) and the penalty math
  (finale) are batched across all tiles; traj/att/out move in single DMAs.
"""
import sys
import types
import numpy as np
from contextlib import ExitStack

NA, T = 256, 100
N_MAPS, MAP_H, MAP_W = 4, 2048, 2048
PU, PV = 10, 20
P = PU * PV  # 200
N_CORES = 8

# jnp.linspace(-0.5, 0.5, 10/20, dtype=float32) exact values (validated vs jax)
_UU10 = np.array([-0.5, -0.3888889, -0.2777778, -0.16666667, -0.05555556,
                  0.05555556, 0.16666667, 0.2777778, 0.3888889, 0.5], dtype=np.float32)
_VV20 = np.linspace(-0.5, 0.5, 20, dtype=np.float32)


def _install_ntff_hook():
    import antenv
    if "antenv.axon_hooks" in sys.modules:
        return
    try:
        from trn_agent_boot.trn_boot import _ntff_profile_via_ctypes
        hook = _ntff_profile_via_ctypes("/opt/axon/libaxon_pjrt.so")
    except Exception:
        hook = None
    mod = types.ModuleType("antenv.axon_hooks")
    mod._hook = hook
    mod.get_axon_ntff_profile_hook = lambda: mod._hook
    mod.set_axon_ntff_profile_hook = lambda h: setattr(mod, "_hook", h)
    sys.modules["antenv.axon_hooks"] = mod
    antenv.axon_hooks = mod


_PROGRAM_CACHE = {}


def _build_program(n_tiles):
    import concourse.tile as tile
    from concourse import bacc, mybir

    dt = mybir.dt
    A = mybir.AluOpType
    NT = n_tiles

    nc = bacc.Bacc("TRN2", target_bir_lowering=False, debug=False,
                   enable_asserts=False, num_devices=N_CORES)

    tab_in = nc.dram_tensor("tab", [128, 16384 * 2], dt.float16, kind="ExternalInput").ap()
    mt2_in = nc.dram_tensor("mt2", [128, 64], dt.float16, kind="ExternalInput").ap()
    w2_in = nc.dram_tensor("w2", [128, 128], dt.float16, kind="ExternalInput").ap()
    uu_in = nc.dram_tensor("uu", [128, P], dt.float32, kind="ExternalInput").ap()
    vv_in = nc.dram_tensor("vv", [128, P], dt.float32, kind="ExternalInput").ap()
    dx_in = nc.dram_tensor("dxrep", [128, 1], dt.float32, kind="ExternalInput").ap()
    mh_in = nc.dram_tensor("maskh", [128, 1], dt.uint8, kind="ExternalInput").ap()
    traj_in = nc.dram_tensor("trajsh", [128, NT * 4], dt.float32, kind="ExternalInput").ap()
    att_in = nc.dram_tensor("attsh", [128, NT * 2], dt.float32, kind="ExternalInput").ap()
    out_dram = nc.dram_tensor("outsh", [128, NT], dt.float32, kind="ExternalOutput").ap()

    with tile.TileContext(nc) as tc, ExitStack() as ctx:
        cpool = ctx.enter_context(tc.tile_pool(name="const", bufs=1))
        wpool = ctx.enter_context(tc.tile_pool(name="work", bufs=1))
        xpool = ctx.enter_context(tc.tile_pool(name="xeng", bufs=2))
        ppool = ctx.enter_context(tc.tile_pool(name="ps", bufs=8, space="PSUM"))

        tab = cpool.tile([128, 16384 * 2], dt.float16)
        nc.sync.dma_start(tab[:], tab_in)
        mt2 = cpool.tile([128, 64], dt.float16)
        nc.sync.dma_start(mt2[:], mt2_in)
        w2 = cpool.tile([128, 128], dt.float16)
        nc.sync.dma_start(w2[:], w2_in)
        uu = cpool.tile([128, P], dt.float32)
        nc.sync.dma_start(uu[:], uu_in)
        vv = cpool.tile([128, P], dt.float32)
        nc.sync.dma_start(vv[:], vv_in)
        dxrep = cpool.tile([128, 1], dt.float32)
        nc.sync.dma_start(dxrep[:], dx_in)
        maskh = cpool.tile([128, 1], dt.uint8)
        nc.sync.dma_start(maskh[:], mh_in)
        trh = cpool.tile([128, NT * 4], dt.float32)
        nc.sync.dma_start(trh[:], traj_in)
        ath = cpool.tile([128, NT * 2], dt.float32)
        nc.sync.dma_start(ath[:], att_in)

        invdx = cpool.tile([128, 1], dt.float32)
        nc.vector.reciprocal(invdx[:], dxrep[:])

        # ---- pre-phase: batched heading normalization over all tiles ----
        cx = trh[:].rearrange("p (t f) -> p t f", f=4)[:, :, 0:1].rearrange("p t f -> p (t f)")
        cy = trh[:].rearrange("p (t f) -> p t f", f=4)[:, :, 1:2].rearrange("p t f -> p (t f)")
        hx0 = trh[:].rearrange("p (t f) -> p t f", f=4)[:, :, 2:3].rearrange("p t f -> p (t f)")
        hy0 = trh[:].rearrange("p (t f) -> p t f", f=4)[:, :, 3:4].rearrange("p t f -> p (t f)")
        Lat = ath[:].rearrange("p (t f) -> p t f", f=2)[:, :, 0:1].rearrange("p t f -> p (t f)")
        Wat = ath[:].rearrange("p (t f) -> p t f", f=2)[:, :, 1:2].rearrange("p t f -> p (t f)")

        pre = cpool.tile([128, NT * 8], dt.float32)
        pv = pre[:].rearrange("p (k t) -> p k t", k=8)
        n1, n2, dl, dn, hx, hy, nhy, inv = (pv[:, k, :] for k in range(8))

        # den = sqrt(hx^2+hy^2) via near-1 expansion (matches baseline numerics)
        nc.vector.tensor_tensor(n1, hx0, hx0, A.mult)
        nc.vector.tensor_tensor(n2, hy0, hy0, A.mult)
        nc.vector.tensor_tensor(n1, n1, n2, A.add)                    # x
        nc.vector.tensor_scalar(dl, n1, -1.0, None, A.add)            # delta
        nc.vector.tensor_scalar(dn, dl, 0.5, None, A.mult)
        nc.vector.tensor_tensor(n2, dl, dl, A.mult)
        nc.vector.tensor_scalar(n2, n2, -0.125, None, A.mult)
        nc.vector.tensor_tensor(dn, dn, n2, A.add)
        nc.vector.tensor_scalar(dn, dn, 1.0, None, A.add)             # den
        nc.vector.reciprocal(inv, dn)
        nc.vector.tensor_tensor(hx, hx0, inv, A.mult)
        nc.vector.tensor_tensor(hy, hy0, inv, A.mult)
        nc.vector.tensor_scalar(nhy, hy, -1.0, None, A.mult)

        MX8 = cpool.tile([128, NT * 8], dt.float32)

        for it in range(n_tiles):
            Lc = Lat[:, it:it + 1]
            Wc = Wat[:, it:it + 1]

            bu = wpool.tile([128, P], dt.float32, tag="bu")
            nc.vector.tensor_scalar(bu[:], uu[:], Lc, None, A.mult)
            bv = wpool.tile([128, P], dt.float32, tag="bv")
            nc.vector.tensor_scalar(bv[:], vv[:], Wc, None, A.mult)
            t1 = wpool.tile([128, P], dt.float32, tag="t1")
            nc.vector.tensor_scalar(t1[:], bu[:], hx[:, it:it + 1], None, A.mult)
            ox = wpool.tile([128, P], dt.float32, tag="ox")
            nc.vector.scalar_tensor_tensor(ox[:], bv[:], nhy[:, it:it + 1], t1[:], A.mult, A.add)
            nc.vector.tensor_scalar(t1[:], bu[:], hy[:, it:it + 1], None, A.mult)
            oy = wpool.tile([128, P], dt.float32, tag="oy")
            nc.vector.scalar_tensor_tensor(oy[:], bv[:], hx[:, it:it + 1], t1[:], A.mult, A.add)

            d2 = wpool.tile([128, P], dt.float32, tag="d2")
            nc.vector.tensor_tensor(d2[:], ox[:], ox[:], A.mult)
            nc.vector.tensor_tensor(t1[:], oy[:], oy[:], A.mult)
            nc.vector.tensor_tensor(d2[:], d2[:], t1[:], A.add)

            ix = wpool.tile([128, P], dt.int32, tag="ix")
            iy = wpool.tile([128, P], dt.int32, tag="iy")
            t3 = wpool.tile([128, P], dt.float32, tag="t3")
            ci = wpool.tile([128, P], dt.int32, tag="ci")
            adi = wpool.tile([128, P], dt.int32, tag="adi")
            for (ov, ctr, res) in ((ox, cx, ix), (oy, cy, iy)):
                # v = (o + ctr) * invdx, clipped; floor via RNE + is_gt correction
                nc.vector.tensor_scalar(t1[:], ov[:], ctr[:, it:it + 1], invdx[:],
                                        A.add, A.mult)
                nc.vector.tensor_scalar(t1[:], t1[:], 0.0, 2047.0, A.max, A.min)
                nc.vector.tensor_copy(ci[:], t1[:])                  # RNE
                nc.vector.tensor_copy(t3[:], ci[:])
                nc.vector.tensor_tensor(t3[:], t3[:], t1[:], A.is_gt)
                nc.vector.tensor_copy(adi[:], t3[:])
                nc.vector.tensor_tensor(res[:], ci[:], adi[:], A.subtract)

            x16 = wpool.tile([128, P], dt.int32, tag="x16")
            nc.vector.tensor_scalar(x16[:], ix[:], 4, None, A.logical_shift_right)
            y8p = wpool.tile([128, P], dt.int32, tag="y8p")
            nc.vector.tensor_scalar(y8p[:], iy[:], 4, None, A.logical_shift_right)
            e32 = wpool.tile([128, P], dt.int32, tag="e32")
            nc.vector.scalar_tensor_tensor(e32[:], x16[:], 128, y8p[:], A.mult, A.add)
            e16 = xpool.tile([128, P], dt.int16, tag="e16")
            nc.vector.tensor_copy(e16[:], e32[:])

            ixm = wpool.tile([128, P], dt.int32, tag="ixm")
            nc.vector.tensor_scalar(ixm[:], ix[:], 15, None, A.bitwise_and)
            sb = wpool.tile([128, P], dt.int32, tag="sb")
            nc.vector.tensor_scalar(sb[:], iy[:], 3, 1, A.logical_shift_right, A.bitwise_and)
            jst2 = wpool.tile([128, P], dt.int32, tag="jst2")
            nc.vector.scalar_tensor_tensor(jst2[:], ixm[:], 2, sb[:], A.mult, A.add)
            j16 = xpool.tile([128, P], dt.int16, tag="j16")
            nc.vector.tensor_copy(j16[:], jst2[:])
            rbit = wpool.tile([128, P], dt.int32, tag="rbit")
            nc.vector.tensor_scalar(rbit[:], iy[:], 7, None, A.bitwise_and)

            g1 = xpool.tile([128, 16 * P * 2], dt.float16, tag="g1")
            nc.gpsimd.ap_gather(g1[:], tab[:], e16[:], channels=128,
                                num_elems=16384, d=2, num_idxs=16 * P)
            g2 = wpool.tile([128, 16 * P * 2], dt.float16, tag="g2")
            nc.gpsimd.ap_gather(g2[:], mt2[:], j16[:], channels=128,
                                num_elems=32, d=2, num_idxs=16 * P)
            nc.vector.tensor_tensor(g1[:], g1[:], g2[:], A.mult)

            # block-diagonal expand-reduce: psum[16g+j, f] = sum_{p in grp g} g1m[p, f]
            stgp = xpool.tile([128, 16 * P * 2], dt.float16, tag="stgp")
            CH = 400
            for c in range(0, 16 * P * 2, CH):
                pt = ppool.tile([128, CH], dt.float32, tag="pt")
                nc.tensor.matmul(pt[:], w2[:], g1[:, c:c + CH], start=True, stop=True)
                nc.scalar.copy(stgp[:, c:c + CH], pt[:])

            # fused pair-add + width-doubling: stg2d[p, s*32+x2*16+j] = w0+w1
            stg2d = xpool.tile([128, 32 * P], dt.float16, tag="stg2d")
            sv = stgp[:].rearrange("p (s j b) -> p s j b", j=16, b=2)
            ev = sv[:, :, :, 0:1].rearrange("p s j b -> p s (j b)").unsqueeze(2) \
                .broadcast_to((128, P, 2, 16))
            odv = sv[:, :, :, 1:2].rearrange("p s j b -> p s (j b)").unsqueeze(2) \
                .broadcast_to((128, P, 2, 16))
            nc.vector.tensor_tensor(
                stg2d[:].rearrange("p (s x j) -> p s x j", x=2, j=16), ev, odv, A.add)

            # DVE 32x32 stream transpose + 2-op merge -> state-major words
            B = wpool.tile([128, 32 * P], dt.float16, tag="B")
            nc.vector.transpose(B[:], stg2d[:])
            wsel = wpool.tile([128, P], dt.float16, tag="wsel")
            bview = B[:].rearrange("p (s y) -> p s y", y=32)
            nc.vector.tensor_copy(wsel[:], bview[:, :, 0:1].rearrange("p s y -> p (s y)"))
            nc.vector.copy_predicated(wsel[:],
                                      maskh[:].broadcast_to((128, P)),
                                      bview[:, :, 16:17].rearrange("p s y -> p (s y)"))

            wi = wpool.tile([128, P], dt.int32, tag="wi")
            nc.vector.tensor_copy(wi[:], wsel[:])
            nc.vector.tensor_tensor(wi[:], wi[:], rbit[:], A.logical_shift_right)
            nc.vector.tensor_scalar(wi[:], wi[:], 1, None, A.bitwise_and)
            key = wpool.tile([128, P], dt.float32, tag="key")
            nc.vector.scalar_tensor_tensor(key[:], wi[:], -1e30, d2[:], A.mult, A.subtract)
            nc.vector.max(MX8[:, it * 8:(it + 1) * 8], key[:])

        # ---- finale: batched penalty math over all tiles ----
        fw = cpool.tile([128, NT * 6], dt.float32)
        fv = fw[:].rearrange("p (k t) -> p k t", k=6)
        mk, ds, rr, pw, ps, vd = (fv[:, k, :] for k in range(6))
        mxv = MX8[:].rearrange("p (t e) -> p t e", e=8)[:, :, 0:1].rearrange("p t e -> p (t e)")
        nc.vector.tensor_scalar(mk, mxv, -1.0, None, A.mult)          # min masked d2
        # dist = sqrt(mk): act sqrt + newton with exact recip
        nc.scalar.activation(ds, mk, mybir.ActivationFunctionType.Sqrt)
        nc.vector.reciprocal(rr, ds)
        nc.vector.tensor_tensor(rr, mk, rr, A.mult)
        nc.vector.tensor_tensor(rr, rr, ds, A.add)
        nc.vector.tensor_scalar(ds, rr, 0.5, None, A.mult)            # dist/dx
        # pen = sqrt(L^2/4 + W^2/4), same refinement
        nc.vector.tensor_tensor(pw, Lat, Lat, A.mult)
        nc.vector.tensor_tensor(ps, Wat, Wat, A.mult)
        nc.vector.tensor_tensor(pw, pw, ps, A.add)
        nc.vector.tensor_scalar(pw, pw, 0.25, None, A.mult)
        nc.scalar.activation(ps, pw, mybir.ActivationFunctionType.Sqrt)
        nc.vector.reciprocal(rr, ps)
        nc.vector.tensor_tensor(rr, pw, rr, A.mult)
        nc.vector.tensor_tensor(rr, rr, ps, A.add)
        nc.vector.tensor_scalar(ps, rr, 0.5, None, A.mult)            # pen
        nc.vector.reciprocal(rr, ps)
        nc.vector.tensor_tensor(ds, ds, rr, A.mult)                   # dist/pen
        nc.vector.tensor_scalar(ds, ds, -1.0, 1.0, A.mult, A.add)     # penalty
        nc.vector.tensor_scalar(vd, mk, 1e29, None, A.is_lt)          # valid
        out_s = cpool.tile([128, NT], dt.float32)
        nc.vector.tensor_tensor(out_s[:], ds, vd, A.mult)
        nc.sync.dma_start(out_dram, out_s[:])

    nc.compile()
    return nc


def kernel(traj, veh_att, raster, mapixes, dx, _trace=False):
    _install_ntff_hook()
    from concourse.bass_utils import run_bass_kernel_spmd

    traj = np.ascontiguousarray(traj, np.float32)
    veh_att = np.ascontiguousarray(veh_att, np.float32)
    raster = np.ascontiguousarray(raster, np.float32)
    mapixes = np.ascontiguousarray(mapixes).astype(np.int64)
    dxf = np.float32(np.asarray(dx).reshape(-1)[0])

    # ---- host layout prep ----
    # Y8 pack: words[m, y8, x] in [0, 256)
    r8 = (raster >= 0.5).astype(np.uint16).reshape(N_MAPS, MAP_H // 8, 8, MAP_W)
    wts = (1 << np.arange(8)).astype(np.uint16)
    words = (r8 * wts[None, None, :, None]).sum(axis=2).astype(np.float16)  # [4,256,2048]

    # per-map ap_gather table [128, 16384, 2]: partition p holds columns
    # x = 16*x16 + (p%16); e = x16*128 + (iy//16); d-pair = y8 words (2*y8p, 2*y8p+1)
    tabs = []
    for m in range(N_MAPS):
        wm = words[m]  # [256, 2048]
        t = np.zeros((128, 16384, 2), np.float16)
        j = (np.arange(128) % 16)
        x16 = np.arange(128)
        y8p = np.arange(128)
        xx = (x16[:, None] * 16)[None, :, :] + j[:, None, None]
        for s in range(2):
            t[:, :, s].reshape(128, 128, 128)[:, :, :] = \
                wm[(y8p * 2 + s)[None, None, :], xx]
        tabs.append(t.reshape(128, 16384 * 2))

    # combined one-hot mask: idx = 2*(ix%16) + sbit; value at lane b:
    #   (p%16 == ix%16) and (b == sbit)
    mt2 = np.zeros((128, 32, 2), np.float16)
    pc = np.arange(128) % 16
    for sbit in range(2):
        mt2[np.arange(128), 2 * pc + sbit, sbit] = 1
    mt2 = mt2.reshape(128, 64)

    # block-diagonal expand-reduce weights: W2[p, o] = (p//16 == o//16)
    w2 = np.zeros((128, 128), np.float16)
    for g in range(8):
        w2[g * 16:(g + 1) * 16, g * 16:(g + 1) * 16] = 1

    uu2, vv2 = np.meshgrid(_UU10, _VV20, indexing="ij")
    uu_rep = np.broadcast_to(uu2.reshape(1, P), (128, P)).astype(np.float32).copy()
    vv_rep = np.broadcast_to(vv2.reshape(1, P), (128, P)).astype(np.float32).copy()
    dxrep = np.full((128, 1), dxf, np.float32)
    maskh = ((np.arange(128) % 32) >= 16).astype(np.uint8).reshape(128, 1)

    # ---- shard agents by map, 2 cores per map ----
    core_agents = [[] for _ in range(N_CORES)]
    for m in range(N_MAPS):
        ags = np.where(mapixes == m)[0]
        half = (len(ags) + 1) // 2
        core_agents[2 * m] = list(ags[:half])
        core_agents[2 * m + 1] = list(ags[half:])

    n_states = [len(a) * T for a in core_agents]
    n_tiles = max(1, int(np.ceil(max(n_states) / 128)))
    S = n_tiles * 128

    in_maps = []
    state_maps = []
    for c in range(N_CORES):
        ags = core_agents[c]
        tr = np.zeros((S, 4), np.float32)
        at = np.zeros((S, 2), np.float32)
        smap = np.full(S, -1, np.int64)
        if ags:
            idx = np.array([(a * T + t) for a in ags for t in range(T)])
            tr[:len(idx)] = traj.reshape(NA * T, 4)[idx]
            at[:len(idx)] = veh_att[np.repeat(ags, T)]
            smap[:len(idx)] = idx
        pad = smap < 0
        tr[pad] = np.array([100.0, 100.0, 1.0, 0.0], np.float32)
        at[pad] = np.array([4.0, 2.0], np.float32)
        # [S,4] -> [128, n_tiles*4] with partition = state-within-tile
        trh = tr.reshape(n_tiles, 128, 4).transpose(1, 0, 2).reshape(128, n_tiles * 4).copy()
        ath = at.reshape(n_tiles, 128, 2).transpose(1, 0, 2).reshape(128, n_tiles * 2).copy()
        in_maps.append({
            "tab": tabs[c // 2], "mt2": mt2, "w2": w2, "uu": uu_rep,
            "vv": vv_rep, "dxrep": dxrep, "maskh": maskh,
            "trajsh": trh, "attsh": ath,
        })
        state_maps.append(smap)

    if n_tiles not in _PROGRAM_CACHE:
        _PROGRAM_CACHE[n_tiles] = _build_program(n_tiles)
    nc = _PROGRAM_CACHE[n_tiles]

    try:
        res = run_bass_kernel_spmd(nc, in_maps, list(range(N_CORES)), trace=_trace)
    except Exception:
        if not _trace:
            raise
        res = run_bass_kernel_spmd(nc, in_maps, list(range(N_CORES)), trace=False)
    kernel.last_results = res

    out = np.zeros(NA * T, np.float32)
    for c in range(N_CORES):
        o = res.results[c]["outsh"]  # [128, n_tiles]
        o = o.T.reshape(-1)          # state k = it*128 + p
        valid = state_maps[c] >= 0
        out[state_maps[c][valid]] = o[valid]
    return out


# revision 12
# speedup vs baseline: 1.9704x; 1.9704x over previous
"""Bass/TRN2 kernel for nn_EnvCollLoss (oriented-footprint raster collision loss).

Strategy: agents sharded by map index across 8 cores (2 cores per map); each
core keeps its map as a Y8-bitpacked fp16 ap_gather table in SBUF.

Per 128-state tile (all on-chip, no DMA in the loop):
  DVE computes footprint-point pixel indices; gpsimd ap_gather fetches the
  16-row word-pair per point on all 16 column-phase partitions of each group,
  plus a combined (column-phase one-hot x pair-half select) mask from a tiny
  table; gpsimd multiplies them; a block-diagonal [128,128] matmul reduces
  each 16-partition group so every partition holds its group's selected word;
  DVE pair-adds the PSUM halves; the scalar engine materializes a width-
  doubled copy; one DVE 32x32 stream-transpose + predicated merge lands the
  words state-major. dist = sqrt(min masked d2), so no collision-point
  reconstruction is needed. Heading normalization (pre) and the penalty math
  (finale) are batched across all tiles; traj/att/out move in single DMAs.
"""
import sys
import types
import numpy as np
from contextlib import ExitStack

NA, T = 256, 100
N_MAPS, MAP_H, MAP_W = 4, 2048, 2048
PU, PV = 10, 20
P = PU * PV  # 200
N_CORES = 8

# jnp.linspace(-0.5, 0.5, 10/20, dtype=float32) exact values (validated vs jax)
_UU10 = np.array([-0.5, -0.3888889, -0.2777778, -0.16666667, -0.05555556,
                  0.05555556, 0.16666667, 0.2777778, 0.3888889, 0.5], dtype=np.float32)
_VV20 = np.linspace(-0.5, 0.5, 20, dtype=np.float32)


def _install_ntff_hook():
    import antenv
    if "antenv.axon_hooks" in sys.modules:
        return
    try:
        from trn_agent_boot.trn_boot import _ntff_profile_via_ctypes
        hook = _ntff_profile_via_ctypes("/opt/axon/libaxon_pjrt.so")
    except Exception:
        hook = None
    mod = types.ModuleType("antenv.axon_hooks")
    mod._hook = hook
    mod.get_axon_ntff_profile_hook = lambda: mod._hook
    mod.set_axon_ntff_profile_hook = lambda h: setattr(mod, "_hook", h)
    sys.modules["antenv.axon_hooks"] = mod
    antenv.axon_hooks = mod


_PROGRAM_CACHE = {}


def _build_program(n_tiles):
    import concourse.tile as tile
    from concourse import bacc, mybir

    dt = mybir.dt
    A = mybir.AluOpType
    NT = n_tiles

    nc = bacc.Bacc("TRN2", target_bir_lowering=False, debug=False,
                   enable_asserts=False, num_devices=N_CORES)

    tab_in = nc.dram_tensor("tab", [128, 16384 * 2], dt.float16, kind="ExternalInput").ap()
    pc20_in = nc.dram_tensor("pc20", [128, 1], dt.float16, kind="ExternalInput").ap()
    pc21_in = nc.dram_tensor("pc21", [128, 1], dt.float16, kind="ExternalInput").ap()
    w2_in = nc.dram_tensor("w2", [128, 128], dt.float16, kind="ExternalInput").ap()
    uu_in = nc.dram_tensor("uu", [128, P], dt.float32, kind="ExternalInput").ap()
    vv_in = nc.dram_tensor("vv", [128, P], dt.float32, kind="ExternalInput").ap()
    dx_in = nc.dram_tensor("dxrep", [128, 1], dt.float32, kind="ExternalInput").ap()
    mh_in = nc.dram_tensor("maskh", [128, 1], dt.uint8, kind="ExternalInput").ap()
    traj_in = nc.dram_tensor("trajsh", [128, NT * 4], dt.float32, kind="ExternalInput").ap()
    att_in = nc.dram_tensor("attsh", [128, NT * 2], dt.float32, kind="ExternalInput").ap()
    out_dram = nc.dram_tensor("outsh", [128, NT], dt.float32, kind="ExternalOutput").ap()

    with tile.TileContext(nc) as tc, ExitStack() as ctx:
        cpool = ctx.enter_context(tc.tile_pool(name="const", bufs=1))
        wpool = ctx.enter_context(tc.tile_pool(name="work", bufs=1))
        xpool = ctx.enter_context(tc.tile_pool(name="xeng", bufs=2))
        spool = ctx.enter_context(tc.tile_pool(name="scr", bufs=2))
        ypool = ctx.enter_context(tc.tile_pool(name="yone", bufs=1))
        ppool = ctx.enter_context(tc.tile_pool(name="ps", bufs=8, space="PSUM"))

        tab = cpool.tile([128, 16384 * 2], dt.float16)
        nc.sync.dma_start(tab[:], tab_in)
        pc20 = cpool.tile([128, 1], dt.float16)
        nc.sync.dma_start(pc20[:], pc20_in)
        pc21 = cpool.tile([128, 1], dt.float16)
        nc.sync.dma_start(pc21[:], pc21_in)
        w2 = cpool.tile([128, 128], dt.float16)
        nc.sync.dma_start(w2[:], w2_in)
        uu = cpool.tile([128, P], dt.float32)
        nc.sync.dma_start(uu[:], uu_in)
        vv = cpool.tile([128, P], dt.float32)
        nc.sync.dma_start(vv[:], vv_in)
        dxrep = cpool.tile([128, 1], dt.float32)
        nc.sync.dma_start(dxrep[:], dx_in)
        maskh = cpool.tile([128, 1], dt.uint8)
        nc.sync.dma_start(maskh[:], mh_in)
        trh = cpool.tile([128, NT * 4], dt.float32)
        nc.sync.dma_start(trh[:], traj_in)
        ath = cpool.tile([128, NT * 2], dt.float32)
        nc.sync.dma_start(ath[:], att_in)

        invdx = cpool.tile([128, 1], dt.float32)
        nc.vector.reciprocal(invdx[:], dxrep[:])

        # ---- pre-phase: batched heading normalization over all tiles ----
        cx = trh[:].rearrange("p (t f) -> p t f", f=4)[:, :, 0:1].rearrange("p t f -> p (t f)")
        cy = trh[:].rearrange("p (t f) -> p t f", f=4)[:, :, 1:2].rearrange("p t f -> p (t f)")
        hx0 = trh[:].rearrange("p (t f) -> p t f", f=4)[:, :, 2:3].rearrange("p t f -> p (t f)")
        hy0 = trh[:].rearrange("p (t f) -> p t f", f=4)[:, :, 3:4].rearrange("p t f -> p (t f)")
        Lat = ath[:].rearrange("p (t f) -> p t f", f=2)[:, :, 0:1].rearrange("p t f -> p (t f)")
        Wat = ath[:].rearrange("p (t f) -> p t f", f=2)[:, :, 1:2].rearrange("p t f -> p (t f)")

        pre = cpool.tile([128, NT * 8], dt.float32)
        pv = pre[:].rearrange("p (k t) -> p k t", k=8)
        n1, n2, dl, dn, hx, hy, nhy, inv = (pv[:, k, :] for k in range(8))

        # den = sqrt(hx^2+hy^2) via near-1 expansion (matches baseline numerics)
        nc.vector.tensor_tensor(n1, hx0, hx0, A.mult)
        nc.vector.tensor_tensor(n2, hy0, hy0, A.mult)
        nc.vector.tensor_tensor(n1, n1, n2, A.add)                    # x
        nc.vector.tensor_scalar(dl, n1, -1.0, None, A.add)            # delta
        nc.vector.tensor_scalar(dn, dl, 0.5, None, A.mult)
        nc.vector.tensor_tensor(n2, dl, dl, A.mult)
        nc.vector.tensor_scalar(n2, n2, -0.125, None, A.mult)
        nc.vector.tensor_tensor(dn, dn, n2, A.add)
        nc.vector.tensor_scalar(dn, dn, 1.0, None, A.add)             # den
        nc.vector.reciprocal(inv, dn)
        nc.vector.tensor_tensor(hx, hx0, inv, A.mult)
        nc.vector.tensor_tensor(hy, hy0, inv, A.mult)
        nc.vector.tensor_scalar(nhy, hy, -1.0, None, A.mult)

        MX8 = cpool.tile([128, NT * 8], dt.float32)

        for it in range(n_tiles):
            Lc = Lat[:, it:it + 1]
            Wc = Wat[:, it:it + 1]

            bu = wpool.tile([128, P], dt.float32, tag="bu")
            nc.vector.tensor_scalar(bu[:], uu[:], Lc, None, A.mult)
            bv = wpool.tile([128, P], dt.float32, tag="bv")
            nc.vector.tensor_scalar(bv[:], vv[:], Wc, None, A.mult)
            t1 = wpool.tile([128, P], dt.float32, tag="t1")
            nc.vector.tensor_scalar(t1[:], bu[:], hx[:, it:it + 1], None, A.mult)
            ox = wpool.tile([128, P], dt.float32, tag="ox")
            nc.vector.scalar_tensor_tensor(ox[:], bv[:], nhy[:, it:it + 1], t1[:], A.mult, A.add)
            nc.vector.tensor_scalar(t1[:], bu[:], hy[:, it:it + 1], None, A.mult)
            oy = wpool.tile([128, P], dt.float32, tag="oy")
            nc.vector.scalar_tensor_tensor(oy[:], bv[:], hx[:, it:it + 1], t1[:], A.mult, A.add)

            d2 = wpool.tile([128, P], dt.float32, tag="d2")
            nc.vector.tensor_tensor(d2[:], ox[:], ox[:], A.mult)
            nc.vector.tensor_tensor(t1[:], oy[:], oy[:], A.mult)
            nc.vector.tensor_tensor(d2[:], d2[:], t1[:], A.add)

            ix = wpool.tile([128, P], dt.int32, tag="ix")
            iy = wpool.tile([128, P], dt.int32, tag="iy")
            t3 = wpool.tile([128, P], dt.float32, tag="t3")
            ci = wpool.tile([128, P], dt.int32, tag="ci")
            adi = wpool.tile([128, P], dt.int32, tag="adi")
            for (ov, ctr, res) in ((ox, cx, ix), (oy, cy, iy)):
                # v = (o + ctr) * invdx, clipped; floor via RNE + is_gt correction
                nc.vector.tensor_scalar(t1[:], ov[:], ctr[:, it:it + 1], invdx[:],
                                        A.add, A.mult)
                nc.vector.tensor_scalar(t1[:], t1[:], 0.0, 2047.0, A.max, A.min)
                nc.vector.tensor_copy(ci[:], t1[:])                  # RNE
                nc.vector.tensor_copy(t3[:], ci[:])
                nc.vector.tensor_tensor(t3[:], t3[:], t1[:], A.is_gt)
                nc.vector.tensor_copy(adi[:], t3[:])
                nc.vector.tensor_tensor(res[:], ci[:], adi[:], A.subtract)

            x16 = wpool.tile([128, P], dt.int32, tag="x16")
            nc.vector.tensor_scalar(x16[:], ix[:], 4, None, A.logical_shift_right)
            y8p = wpool.tile([128, P], dt.int32, tag="y8p")
            nc.vector.tensor_scalar(y8p[:], iy[:], 4, None, A.logical_shift_right)
            e32 = wpool.tile([128, P], dt.int32, tag="e32")
            nc.vector.scalar_tensor_tensor(e32[:], x16[:], 128, y8p[:], A.mult, A.add)
            e16 = xpool.tile([128, P], dt.int16, tag="e16")
            nc.vector.tensor_copy(e16[:], e32[:])

            ixm = wpool.tile([128, P], dt.int32, tag="ixm")
            nc.vector.tensor_scalar(ixm[:], ix[:], 15, None, A.bitwise_and)
            sb = wpool.tile([128, P], dt.int32, tag="sb")
            nc.vector.tensor_scalar(sb[:], iy[:], 3, 1, A.logical_shift_right, A.bitwise_and)
            jst2 = wpool.tile([128, P], dt.int32, tag="jst2")
            nc.vector.scalar_tensor_tensor(jst2[:], ixm[:], 2, sb[:], A.mult, A.add)
            jsth = wpool.tile([128, P], dt.float16, tag="jsth")
            nc.vector.tensor_copy(jsth[:], jst2[:])
            rbit = wpool.tile([128, P], dt.int32, tag="rbit")
            nc.vector.tensor_scalar(rbit[:], iy[:], 7, None, A.bitwise_and)

            g1 = xpool.tile([128, 16 * P * 2], dt.float16, tag="g1")
            nc.gpsimd.ap_gather(g1[:], tab[:], e16[:], channels=128,
                                num_elems=16384, d=2, num_idxs=16 * P)

            # mask via stream-transpose of jst (no second gather):
            # Tj[32a+x, s*32+y] = jst[32a+y, s]; merge halves -> U[p, s*16+j]
            Tj = spool.tile([128, 32 * P], dt.float16, tag="scr")
            nc.vector.transpose(
                Tj[:], jsth[:].unsqueeze(2).broadcast_to((128, P, 32)))
            U = wpool.tile([128, 16 * P], dt.float16, tag="U")
            tjv = Tj[:].rearrange("p (s y) -> p s y", y=32)
            nc.vector.tensor_copy(
                U[:].rearrange("p (s j) -> p s j", j=16), tjv[:, :, 0:16])
            nc.vector.copy_predicated(
                U[:].rearrange("p (s j) -> p s j", j=16),
                maskh[:].unsqueeze(2).broadcast_to((128, P, 16)), tjv[:, :, 16:32])

            # masked select: g1m lane b = (U == 2*(p%16)+b) * g1 lane b
            g1m = ypool.tile([128, 16 * P * 2], dt.float16, tag="g1m")
            g1v = g1[:].rearrange("p (s b) -> p s b", b=2)
            gmv = g1m[:].rearrange("p (s b) -> p s b", b=2)
            for b, pcb in ((0, pc20), (1, pc21)):
                nc.vector.scalar_tensor_tensor(
                    gmv[:, :, b:b + 1].rearrange("p s b -> p (s b)"), U[:], pcb[:],
                    g1v[:, :, b:b + 1].rearrange("p s b -> p (s b)"),
                    A.is_equal, A.mult)

            # block-diagonal expand-reduce: psum[16g+j, f] = sum_{p in grp g} g1m[p, f]
            stgp = ypool.tile([128, 16 * P * 2], dt.float16, tag="stgp")
            CH = 400
            for c in range(0, 16 * P * 2, CH):
                pt = ppool.tile([128, CH], dt.float32, tag="pt")
                nc.tensor.matmul(pt[:], w2[:], g1m[:, c:c + CH], start=True, stop=True)
                nc.scalar.copy(stgp[:, c:c + CH], pt[:])

            # fused pair-add + width-doubling: stg2d[p, s*32+x2*16+j] = w0+w1
            stg2d = xpool.tile([128, 32 * P], dt.float16, tag="stg2d")
            sv = stgp[:].rearrange("p (s j b) -> p s j b", j=16, b=2)
            ev = sv[:, :, :, 0:1].rearrange("p s j b -> p s (j b)").unsqueeze(2) \
                .broadcast_to((128, P, 2, 16))
            odv = sv[:, :, :, 1:2].rearrange("p s j b -> p s (j b)").unsqueeze(2) \
                .broadcast_to((128, P, 2, 16))
            nc.vector.tensor_tensor(
                stg2d[:].rearrange("p (s x j) -> p s x j", x=2, j=16), ev, odv, A.add)

            # DVE 32x32 stream transpose + 2-op merge -> state-major words
            B = spool.tile([128, 32 * P], dt.float16, tag="scr")
            nc.vector.transpose(B[:], stg2d[:])
            wsel = wpool.tile([128, P], dt.float16, tag="wsel")
            bview = B[:].rearrange("p (s y) -> p s y", y=32)
            nc.vector.tensor_copy(wsel[:], bview[:, :, 0:1].rearrange("p s y -> p (s y)"))
            nc.vector.copy_predicated(wsel[:],
                                      maskh[:].broadcast_to((128, P)),
                                      bview[:, :, 16:17].rearrange("p s y -> p (s y)"))

            wi = wpool.tile([128, P], dt.int32, tag="wi")
            nc.vector.tensor_copy(wi[:], wsel[:])
            nc.vector.tensor_tensor(wi[:], wi[:], rbit[:], A.logical_shift_right)
            nc.vector.tensor_scalar(wi[:], wi[:], 1, None, A.bitwise_and)
            key = wpool.tile([128, P], dt.float32, tag="key")
            nc.vector.scalar_tensor_tensor(key[:], wi[:], -1e30, d2[:], A.mult, A.subtract)
            nc.vector.max(MX8[:, it * 8:(it + 1) * 8], key[:])

        # ---- finale: batched penalty math over all tiles ----
        fw = cpool.tile([128, NT * 6], dt.float32)
        fv = fw[:].rearrange("p (k t) -> p k t", k=6)
        mk, ds, rr, pw, ps, vd = (fv[:, k, :] for k in range(6))
        mxv = MX8[:].rearrange("p (t e) -> p t e", e=8)[:, :, 0:1].rearrange("p t e -> p (t e)")
        nc.vector.tensor_scalar(mk, mxv, -1.0, None, A.mult)          # min masked d2
        # dist = sqrt(mk): act sqrt + newton with exact recip
        nc.scalar.activation(ds, mk, mybir.ActivationFunctionType.Sqrt)
        nc.vector.reciprocal(rr, ds)
        nc.vector.tensor_tensor(rr, mk, rr, A.mult)
        nc.vector.tensor_tensor(rr, rr, ds, A.add)
        nc.vector.tensor_scalar(ds, rr, 0.5, None, A.mult)            # dist/dx
        # pen = sqrt(L^2/4 + W^2/4), same refinement
        nc.vector.tensor_tensor(pw, Lat, Lat, A.mult)
        nc.vector.tensor_tensor(ps, Wat, Wat, A.mult)
        nc.vector.tensor_tensor(pw, pw, ps, A.add)
        nc.vector.tensor_scalar(pw, pw, 0.25, None, A.mult)
        nc.scalar.activation(ps, pw, mybir.ActivationFunctionType.Sqrt)
        nc.vector.reciprocal(rr, ps)
        nc.vector.tensor_tensor(rr, pw, rr, A.mult)
        nc.vector.tensor_tensor(rr, rr, ps, A.add)
        nc.vector.tensor_scalar(ps, rr, 0.5, None, A.mult)            # pen
        nc.vector.reciprocal(rr, ps)
        nc.vector.tensor_tensor(ds, ds, rr, A.mult)                   # dist/pen
        nc.vector.tensor_scalar(ds, ds, -1.0, 1.0, A.mult, A.add)     # penalty
        nc.vector.tensor_scalar(vd, mk, 1e29, None, A.is_lt)          # valid
        out_s = cpool.tile([128, NT], dt.float32)
        nc.vector.tensor_tensor(out_s[:], ds, vd, A.mult)
        nc.sync.dma_start(out_dram, out_s[:])

    nc.compile()
    return nc


def kernel(traj, veh_att, raster, mapixes, dx, _trace=False):
    _install_ntff_hook()
    from concourse.bass_utils import run_bass_kernel_spmd

    traj = np.ascontiguousarray(traj, np.float32)
    veh_att = np.ascontiguousarray(veh_att, np.float32)
    raster = np.ascontiguousarray(raster, np.float32)
    mapixes = np.ascontiguousarray(mapixes).astype(np.int64)
    dxf = np.float32(np.asarray(dx).reshape(-1)[0])

    # ---- host layout prep ----
    # Y8 pack: words[m, y8, x] in [0, 256)
    r8 = (raster >= 0.5).astype(np.uint16).reshape(N_MAPS, MAP_H // 8, 8, MAP_W)
    wts = (1 << np.arange(8)).astype(np.uint16)
    words = (r8 * wts[None, None, :, None]).sum(axis=2).astype(np.float16)  # [4,256,2048]

    # per-map ap_gather table [128, 16384, 2]: partition p holds columns
    # x = 16*x16 + (p%16); e = x16*128 + (iy//16); d-pair = y8 words (2*y8p, 2*y8p+1)
    tabs = []
    for m in range(N_MAPS):
        wm = words[m]  # [256, 2048]
        t = np.zeros((128, 16384, 2), np.float16)
        j = (np.arange(128) % 16)
        x16 = np.arange(128)
        y8p = np.arange(128)
        xx = (x16[:, None] * 16)[None, :, :] + j[:, None, None]
        for s in range(2):
            t[:, :, s].reshape(128, 128, 128)[:, :, :] = \
                wm[(y8p * 2 + s)[None, None, :], xx]
        tabs.append(t.reshape(128, 16384 * 2))

    pc20 = (2.0 * (np.arange(128) % 16)).astype(np.float16).reshape(128, 1)
    pc21 = pc20 + np.float16(1.0)

    # block-diagonal expand-reduce weights: W2[p, o] = (p//16 == o//16)
    w2 = np.zeros((128, 128), np.float16)
    for g in range(8):
        w2[g * 16:(g + 1) * 16, g * 16:(g + 1) * 16] = 1

    uu2, vv2 = np.meshgrid(_UU10, _VV20, indexing="ij")
    uu_rep = np.broadcast_to(uu2.reshape(1, P), (128, P)).astype(np.float32).copy()
    vv_rep = np.broadcast_to(vv2.reshape(1, P), (128, P)).astype(np.float32).copy()
    dxrep = np.full((128, 1), dxf, np.float32)
    maskh = ((np.arange(128) % 32) >= 16).astype(np.uint8).reshape(128, 1)

    # ---- shard agents by map, 2 cores per map ----
    core_agents = [[] for _ in range(N_CORES)]
    for m in range(N_MAPS):
        ags = np.where(mapixes == m)[0]
        half = (len(ags) + 1) // 2
        core_agents[2 * m] = list(ags[:half])
        core_agents[2 * m + 1] = list(ags[half:])

    n_states = [len(a) * T for a in core_agents]
    n_tiles = max(1, int(np.ceil(max(n_states) / 128)))
    S = n_tiles * 128

    in_maps = []
    state_maps = []
    for c in range(N_CORES):
        ags = core_agents[c]
        tr = np.zeros((S, 4), np.float32)
        at = np.zeros((S, 2), np.float32)
        smap = np.full(S, -1, np.int64)
        if ags:
            idx = np.array([(a * T + t) for a in ags for t in range(T)])
            tr[:len(idx)] = traj.reshape(NA * T, 4)[idx]
            at[:len(idx)] = veh_att[np.repeat(ags, T)]
            smap[:len(idx)] = idx
        pad = smap < 0
        tr[pad] = np.array([100.0, 100.0, 1.0, 0.0], np.float32)
        at[pad] = np.array([4.0, 2.0], np.float32)
        # [S,4] -> [128, n_tiles*4] with partition = state-within-tile
        trh = tr.reshape(n_tiles, 128, 4).transpose(1, 0, 2).reshape(128, n_tiles * 4).copy()
        ath = at.reshape(n_tiles, 128, 2).transpose(1, 0, 2).reshape(128, n_tiles * 2).copy()
        in_maps.append({
            "tab": tabs[c // 2], "pc20": pc20, "pc21": pc21, "w2": w2, "uu": uu_rep,
            "vv": vv_rep, "dxrep": dxrep, "maskh": maskh,
            "trajsh": trh, "attsh": ath,
        })
        state_maps.append(smap)

    if n_tiles not in _PROGRAM_CACHE:
        _PROGRAM_CACHE[n_tiles] = _build_program(n_tiles)
    nc = _PROGRAM_CACHE[n_tiles]

    try:
        res = run_bass_kernel_spmd(nc, in_maps, list(range(N_CORES)), trace=_trace)
    except Exception:
        if not _trace:
            raise
        res = run_bass_kernel_spmd(nc, in_maps, list(range(N_CORES)), trace=False)
    kernel.last_results = res

    out = np.zeros(NA * T, np.float32)
    for c in range(N_CORES):
        o = res.results[c]["outsh"]  # [128, n_tiles]
        o = o.T.reshape(-1)          # state k = it*128 + p
        valid = state_maps[c] >= 0
        out[state_maps[c][valid]] = o[valid]
    return out
